# revision 4
# baseline (speedup 1.0000x reference)
"""Trainium2 Bass kernel for a dense transformer encoder layer (v2).

Problem (hardcoded): x [2, 2048, 1024], 16 heads, FFN 4096, fp32,
post-LN residual blocks, mask additively applied before softmax.

Sharding: sequence-parallel over the 4096 tokens -> 512 queries per core
(cores 0-3 handle batch 0, cores 4-7 batch 1). Collectives are broken on
this stack, so every core computes the full-batch K/V projections itself.

v2 design (vs the DRAM-bounce baseline):
  * All matmul operands bf16 (weights + x converted on the host). Same
    PE rate as fp32r (1 cycle/row) but half the SBUF/DMA, which lets
    K^T and V' live entirely in SBUF -- no DRAM bounce.
  * K-projection m-tiles are interleaved with attention head-pairs in
    emission order so the PE instruction stream has no gap: the HAM
    clock gate stays at 2.4 GHz (the old kernel spent 430 us at 1.2).
  * Softmax: denominator from a ones-column in V' (row 64 of the
    attn@V' accumulator); reciprocal on DVE, broadcast to 64 partitions
    on the idle GpSimd engine, one DVE multiply to normalize.
  * bv is folded into the projection bias on the host (bp' = bp+bv@Wp);
    LayerNorm uses 1/std = exp(-0.5*ln(var)) so every activation
    (Exp/Square/Ln/Relu/Identity) comes from ONE table -- no
    ACT_TABLE_LOAD thrash when phases interleave.
  * proj/FFN2/V use free-dim-1024 moving operands (legal for bf16);
    w2 is resident in SBUF, FFN1 streams w1; FFN2 per-q-tile
    accumulation paces behind FFN1 via hT dependencies.

Matmul layouts (out = lhsT.T @ rhs, contraction on the partition dim):
  Q^T/K^T : lhsT = W k/m-tile [din,dout], rhs = x^T [din,tok]  -> [dout,tok]
  V       : lhsT = x^T [din,tok],  rhs = Wv [din,dout]         -> [tok,dout]
  scoresT : lhsT = K^T head [dh,kpos], rhs = Q^T head [dh,qpos]-> [kpos,qpos]
  attn@V' : lhsT = V' [kpos,dh+1], rhs = expT [kpos,qpos]      -> [dh+1,qpos]
            (V' has a ones column -> row dh is the softmax denominator)
  outproj : lhsT = o^T [din,q], rhs = Wp [din,dout]            -> [q,dout]
  FFN1    : lhsT = W1 [din,dffn], rhs = xln1^T [din,q]         -> [dffn,q]
  FFN2    : lhsT = h^T [dffn,q], rhs = W2 [dffn,dout]          -> [q,dout]
"""

import numpy as np

import concourse.bass as bass
import concourse.mybir as mybir
import concourse.tile as tile
from concourse.bass_utils import run_bass_kernel_spmd
from concourse.masks import make_identity
from concourse.vector_clock import ScopedClock

FP32 = mybir.dt.float32
BF16 = mybir.dt.bfloat16
AF = mybir.ActivationFunctionType
ALU = mybir.AluOpType

P = 128
D = 1024
F = 4096
H = 16
DH = 64
S = 2048          # tokens per batch
TPC = 512         # tokens (queries) per core
NB = D // P       # 8 dout blocks
KB = D // P       # 8 contraction tiles over D
FB = F // P       # 32 dffn tiles
QT = TPC // P     # 4 query tiles
KT16 = S // P     # 16 kpos tiles
NG = S // TPC     # 4 kpos 512-slices
VW = H * (DH + 1)  # 1040: V' row width per kpos tile
SCALE = DH ** -0.5
N_CORES = 8


# --- Tile tail-drain fix: this walrus build allows only one sem-wait per
# instruction; Tile's final drain accumulates several. Split them across
# dedicated nops before draining.
def _patched_drain_and_barrier(self, tick_clock, wait_clock):
    probe = self.nc.sync.nop(nofuse=True, hint="drain_wait_split")
    wait_clock.add_sem_waits(probe.ins, ScopedClock({None: tick_clock.global_clock}))
    si = probe.ins.sync_info
    if si is not None and si.on_wait and len(si.on_wait) > 1:
        waits = list(si.on_wait)
        si.on_wait = waits[:1]
        for w in waits[1:]:
            extra = self.nc.sync.nop(nofuse=True, hint="drain_wait_split")
            esi = extra.ins.sync_info
            if esi is None:
                extra.ins.sync_info = mybir.SyncInfo(on_wait=[w], on_update=[])
            else:
                esi.on_wait = [w]
    self.nc.sync.drain()
    self.nc.all_engine_barrier()
    assert self.sems is not None
    popped = self.nc._tile_sem_poison_stack.pop()
    assert popped is self._sem_poison
    self.nc.clear_and_free_semaphores(list(self.sems.allocated().values()))
    self.nc.all_engine_barrier()


if getattr(tile.TileContext, "_drain_patch", None) is None:
    tile.TileContext._drain_and_barrier = _patched_drain_and_barrier
    tile.TileContext._drain_patch = True


def _split_waits(nc):
    """Walrus codegen accepts at most one sem-wait per instruction (two on
    EventSemaphore). Tile's scheduler can emit more; hoist the surplus onto
    same-engine EventSemaphore instructions inserted just before."""
    uid = [0]
    for bb in nc.m.functions[0].blocks:
        new_insts = []
        for inst in bb.instructions:
            si = inst.sync_info
            limit = 2 if isinstance(inst, mybir.InstEventSemaphore) else 1
            if si is not None and si.on_wait and len(si.on_wait) > limit:
                waits = list(si.on_wait)
                extra, keep = waits[:-limit], waits[-limit:]
                for i in range(0, len(extra), 2):
                    uid[0] += 1
                    ev = mybir.InstEventSemaphore(
                        name=f"I-wsplit-{uid[0]}",
                        engine=inst.engine,
                        sync_info=mybir.SyncInfo(
                            on_wait=extra[i:i + 2], on_update=[]),
                    )
                    nc.register_instruction(ev)
                    new_insts.append(ev)
                si.on_wait = keep
            new_insts.append(inst)
        if len(new_insts) != len(bb.instructions):
            bb.instructions[:] = new_insts


def _ln_chain(nc, scr, y, out_ap, gamma_b, beta_b=None):
    """LayerNorm over the free dim of y [128, D] fp32 (torch semantics:
    unbiased std; the +eps on std is dropped -- eps=1e-6 against std~1).
    1/std = exp(-0.5*ln(ss/(D-1))) keeps everything in ONE act table.
    y is clobbered (used as Square scratch)."""
    s1 = scr.tile([P, 1], FP32, tag="ln_s1")
    nc.vector.reduce_sum(s1[:], y, axis=mybir.AxisListType.X)
    mnn = scr.tile([P, 1], FP32, tag="ln_mnn")
    nc.scalar.mul(mnn[:], s1[:], -1.0 / D)
    cen = scr.tile([P, D], FP32, tag="ln_cen")
    nc.scalar.activation(cen[:], y, AF.Identity, bias=mnn[:])
    ss = scr.tile([P, 1], FP32, tag="ln_ss")
    nc.scalar.activation(y, cen[:], AF.Square, accum_out=ss[:])
    lt = scr.tile([P, 1], FP32, tag="ln_lt")
    nc.scalar.activation(lt[:], ss[:], AF.Ln, scale=1.0 / (D - 1))
    inv = scr.tile([P, 1], FP32, tag="ln_inv")
    nc.scalar.activation(inv[:], lt[:], AF.Exp, scale=-0.5)
    nc.vector.scalar_tensor_tensor(
        out_ap, cen[:], inv[:], gamma_b, op0=ALU.mult, op1=ALU.mult
    )
    if beta_b is not None:
        nc.vector.tensor_add(out_ap, out_ap, beta_b)


def build_program(use_mask: bool) -> bass.Bass:
    nc = bass.Bass(target_bir_lowering=False, debug=False)

    # ---- I/O ----
    xT_d = nc.dram_tensor("xT", [D, S], BF16, kind="ExternalInput")
    xTq_d = nc.dram_tensor("xTq", [D, TPC], BF16, kind="ExternalInput")
    xbp_d = nc.dram_tensor("xbp", [TPC, D], FP32, kind="ExternalInput")
    wq_d = nc.dram_tensor("wq", [D, D], BF16, kind="ExternalInput")
    wk_d = nc.dram_tensor("wk", [D, D], BF16, kind="ExternalInput")
    wv_d = nc.dram_tensor("wv", [D, D], BF16, kind="ExternalInput")
    wp_d = nc.dram_tensor("wp", [D, D], BF16, kind="ExternalInput")
    w1_d = nc.dram_tensor("w1", [D, F], BF16, kind="ExternalInput")
    w2_d = nc.dram_tensor("w2", [F, D], BF16, kind="ExternalInput")
    bq_d = nc.dram_tensor("bq", [D], FP32, kind="ExternalInput")
    bk_d = nc.dram_tensor("bk", [D], FP32, kind="ExternalInput")
    b1_d = nc.dram_tensor("b1", [F], FP32, kind="ExternalInput")
    b2_d = nc.dram_tensor("b2", [D], FP32, kind="ExternalInput")
    g1_d = nc.dram_tensor("g1", [D], FP32, kind="ExternalInput")
    be1_d = nc.dram_tensor("be1", [D], FP32, kind="ExternalInput")
    g2_d = nc.dram_tensor("g2", [D], FP32, kind="ExternalInput")
    be2_d = nc.dram_tensor("be2", [D], FP32, kind="ExternalInput")
    if use_mask:
        maskT_d = nc.dram_tensor("maskT", [S, TPC], BF16, kind="ExternalInput")
    out_d = nc.dram_tensor("out", [TPC, D], FP32, kind="ExternalOutput")

    with tile.TileContext(nc) as tc:
        with tc.tile_pool(name="dram", bufs=2, space="DRAM") as dramp:
            _build_body(
                nc, tc, use_mask, dramp,
                xT_d, xTq_d, xbp_d, wq_d, wk_d, wv_d, wp_d, w1_d, w2_d,
                bq_d, bk_d, b1_d, b2_d, g1_d, be1_d, g2_d, be2_d,
                maskT_d if use_mask else None, out_d,
            )
    _split_waits(nc)
    return nc


def _build_body(nc, tc, use_mask, dramp, xT_d, xTq_d, xbp_d, wq_d, wk_d, wv_d,
                wp_d, w1_d, w2_d, bq_d, bk_d, b1_d, b2_d,
                g1_d, be1_d, g2_d, be2_d, maskT_d, out_d):
    from contextlib import ExitStack

    with ExitStack() as top:
        consts = top.enter_context(tc.tile_pool(name="consts", bufs=1))
        ident = consts.tile([P, P], FP32)
        make_identity(nc, ident[:])
        bq_c = consts.tile([P, NB], FP32)
        nc.sync.dma_start(bq_c[:], bq_d.ap().rearrange("(b p) -> p b", p=P))
        bk_c = consts.tile([P, NB], FP32)
        nc.sync.dma_start(bk_c[:], bk_d.ap().rearrange("(b p) -> p b", p=P))
        b1_c = consts.tile([P, FB], FP32)
        nc.sync.dma_start(b1_c[:], b1_d.ap().rearrange("(b p) -> p b", p=P))
        be1_c = consts.tile([P, NB], FP32)
        nc.sync.dma_start(be1_c[:], be1_d.ap().rearrange("(b p) -> p b", p=P))
        ones16 = consts.tile([P, H], BF16)
        nc.vector.memset(ones16[:], 1.0)
        if use_mask:
            identb = consts.tile([P, P], BF16)
            nc.vector.tensor_copy(identb[:], ident[:])

        # long-lived LN tensors (strict LIFO pool discipline: open before
        # everything that closes earlier)
        bc1 = top.enter_context(tc.tile_pool(name="bc1", bufs=1))
        g1_b = bc1.tile([P, D], FP32, name="g1b")
        nc.sync.dma_start(g1_b[:], g1_d.ap()[None, :].to_broadcast((P, D)))
        be1_b = bc1.tile([P, D], FP32, name="be1b")
        nc.sync.dma_start(be1_b[:], be1_d.ap()[None, :].to_broadcast((P, D)))
        lnp = top.enter_context(tc.tile_pool(name="lnp", bufs=1))
        xln1 = lnp.tile([P, QT * D], FP32)        # LN1 out (natural), 16KB
        xln1T = lnp.tile([P, KB * TPC], BF16)     # its transpose, 8KB

        # proj-era pools that outlive the QKV scope (ot written by
        # attention inside it; wp rides the QKV weight pool so its DMA
        # lands early)
        proj_era = top.enter_context(ExitStack())
        otp = proj_era.enter_context(tc.tile_pool(name="otp", bufs=1))
        ot_sb = otp.tile([P, NB * TPC], BF16)     # o^T,  8KB/part
        wpl = proj_era.enter_context(tc.tile_pool(name="wql", bufs=2))

        # attention-era pools: closed right after the interleaved
        # QKV+attention block so later pools can reuse their space
        attn_era = top.enter_context(ExitStack())
        kvp = attn_era.enter_context(tc.tile_pool(name="kv", bufs=1))
        kt_sb = kvp.tile([P, NB * S], BF16)       # K^T, 32KB/part
        vp_sb = kvp.tile([P, KT16 * VW], BF16)    # V',  32.5KB/part
        qt_sb = kvp.tile([P, NB * TPC], BF16)     # Q^T,  8KB/part
        ascr = attn_era.enter_context(
            tc.tile_pool(name="ascr", bufs=1 if use_mask else 2))
        sps = attn_era.enter_context(
            tc.tile_pool(name="sps", bufs=2, space="PSUM"))
        ops = attn_era.enter_context(
            tc.tile_pool(name="ops", bufs=2, space="PSUM"))
        if use_mask:
            msk = attn_era.enter_context(tc.tile_pool(name="msk", bufs=2))
        mk_pool = msk if use_mask else None

        # ================= QKV projections =================
        with (
            tc.tile_pool(name="xtp", bufs=1) as xtp,
            tc.tile_pool(name="psA", bufs=2, space="PSUM") as psA,
        ):
            # PE warmup: the HAM clock gate needs ~3.4us of sustained PE
            # activity to release 2.4 GHz. The first real matmul waits on
            # ~8MB of DMA; fill that window with dummy matmuls so the
            # array is warm when real work arrives.
            wrm = xtp.tile([P, TPC], BF16, tag="warm")
            nc.vector.memset(wrm[:], 0.0)
            wps = psA.tile([P, TPC], FP32, tag="ps", name="warm_ps")
            for i in range(48):
                nc.tensor.matmul(wps[:], lhsT=wrm[:, 0:P], rhs=wrm[:],
                                 start=True, stop=True)

            w_q = wpl.tile([P, KB * D], BF16, tag="w")
            xtq = xtp.tile([P, KB * TPC], BF16, tag="xtq")
            for k in range(KB):
                nc.sync.dma_start(w_q[:, k * D:(k + 1) * D],
                                  wq_d.ap()[k * P:(k + 1) * P, :])
                nc.sync.dma_start(xtq[:, k * TPC:(k + 1) * TPC],
                                  xTq_d.ap()[k * P:(k + 1) * P, :])
            xt = xtp.tile([P, KB * S], BF16, tag="xt")
            for k in range(KB):
                nc.scalar.dma_start(xt[:, k * S:(k + 1) * S],
                                    xT_d.ap()[k * P:(k + 1) * P, :])
            w_v = wpl.tile([P, KB * D], BF16, tag="w")
            for k in range(KB):
                nc.scalar.dma_start(w_v[:, k * D:(k + 1) * D],
                                    wv_d.ap()[k * P:(k + 1) * P, :])

            # --- Q^T (+bq) ---
            for m in range(NB):
                ps = psA.tile([P, TPC], FP32, tag="ps")
                for k in range(KB):
                    nc.tensor.matmul(
                        ps[:],
                        lhsT=w_q[:, k * D + m * P: k * D + (m + 1) * P],
                        rhs=xtq[:, k * TPC:(k + 1) * TPC],
                        start=(k == 0), stop=(k == KB - 1),
                    )
                nc.vector.tensor_scalar_add(
                    qt_sb[:, m * TPC:(m + 1) * TPC], ps[:], bq_c[:, m:m + 1])

            for i in range(36):
                nc.tensor.matmul(wps[:], lhsT=wrm[:, 0:P], rhs=wrm[:],
                                 start=True, stop=True)

            # --- V -> V' in SBUF (no bias; bv folded into bp' on host) ---
            for mt in range(KT16):
                v3 = vp_sb[:, mt * VW:(mt + 1) * VW].rearrange(
                    "p (h j) -> p h j", j=DH + 1)
                nc.vector.tensor_copy(v3[:, :, DH], ones16[:])
                for nd in range(2):
                    ps = psA.tile([P, TPC], FP32, tag="ps")
                    for k in range(KB):
                        nc.tensor.matmul(
                            ps[:],
                            lhsT=xt[:, k * S + mt * P: k * S + (mt + 1) * P],
                            rhs=w_v[:, k * D + nd * TPC: k * D + (nd + 1) * TPC],
                            start=(k == 0), stop=(k == KB - 1),
                        )
                    nc.vector.tensor_copy(
                        v3[:, nd * (H // 2):(nd + 1) * (H // 2), 0:DH],
                        ps[:].rearrange("p (h j) -> p h j", j=DH))

            w_k = wpl.tile([P, KB * D], BF16, tag="w")
            for k in range(KB):
                nc.scalar.dma_start(w_k[:, k * D:(k + 1) * D],
                                    wk_d.ap()[k * P:(k + 1) * P, :])

            # --- K^T m-tiles interleaved with attention head-pairs ---
            for m in range(NB):
                for ng in range(NG):
                    ps = psA.tile([P, TPC], FP32, tag="ps")
                    for k in range(KB):
                        nc.tensor.matmul(
                            ps[:],
                            lhsT=w_k[:, k * D + m * P: k * D + (m + 1) * P],
                            rhs=xt[:, k * S + ng * TPC: k * S + (ng + 1) * TPC],
                            start=(k == 0), stop=(k == KB - 1),
                        )
                    nc.vector.tensor_scalar_add(
                        kt_sb[:, m * S + ng * TPC: m * S + (ng + 1) * TPC],
                        ps[:], bk_c[:, m:m + 1])
                _attn_heads(nc, use_mask, 2 * m, 2 * m + 2, kt_sb, qt_sb,
                            vp_sb, ot_sb, ascr, sps, ops, dramp,
                            mk_pool, maskT_d,
                            identb if use_mask else None)
            # wp into the weight pool now: its DMA lands mid-era (WAR on
            # the V matmuls only), so proj can start during late attention
            wp_sb = wpl.tile([P, KB * D], BF16, tag="w")
            nc.sync.dma_start(wp_sb[:].rearrange("p (k n) -> p k n", n=D),
                              wp_d.ap().rearrange("(k p) n -> p k n", p=P))

        # attention-era SBUF/PSUM no longer needed past this point
        attn_era.close()

        # ================= stage-C tensors ========
        prj = proj_era.enter_context(tc.tile_pool(name="prj", bufs=1))
        xbp_sb = prj.tile([P, QT * D], FP32)
        nc.sync.dma_start(xbp_sb[:].rearrange("p (q n) -> p q n", n=D),
                          xbp_d.ap().rearrange("(q p) n -> p q n", p=P))

        # ================= out-proj + LN1 + transpose =================
        with (
            tc.tile_pool(name="pjp", bufs=1, space="PSUM") as pjp,
            tc.tile_pool(name="tpp", bufs=2, space="PSUM") as tpp,
            tc.tile_pool(name="flp", bufs=1, space="PSUM") as flp,
            tc.tile_pool(name="pscr", bufs=2) as pscr,
        ):
            # HAM pinning: the proj/LN1 era has short PE gaps (attention
            # tail chain, LN latency) that drop the clock gate to 1.2 GHz;
            # dep-free filler matmuls keep the activity window busy.
            flw = pscr.tile([P, TPC], BF16, tag="flw")
            nc.vector.memset(flw[:], 0.0)
            flq = flp.tile([P, TPC], FP32, tag="flq", name="fill_ps")

            def _fill(n):
                for _ in range(n):
                    nc.tensor.matmul(flq[:], lhsT=flw[:, 0:P], rhs=flw[:],
                                     start=True, stop=True)

            _fill(16)
            for qt in range(QT):
                pj = [pjp.tile([P, TPC], FP32, tag="pj", name=f"pj_{qt}_{nd}")
                      for nd in range(2)]
                for k in range(KB):
                    for nd in range(2):
                        nc.tensor.matmul(
                            pj[nd][:],
                            lhsT=ot_sb[:, k * TPC + qt * P:
                                       k * TPC + (qt + 1) * P],
                            rhs=wp_sb[:, k * D + nd * TPC:
                                      k * D + (nd + 1) * TPC],
                            start=(k == 0), stop=(k == KB - 1),
                        )
                ysc = pscr.tile([P, D], FP32, tag="ysc")
                for nd in range(2):
                    nc.vector.tensor_add(
                        ysc[:, nd * TPC:(nd + 1) * TPC], pj[nd][:],
                        xbp_sb[:, qt * D + nd * TPC: qt * D + (nd + 1) * TPC])
                _ln_chain(nc, pscr, ysc[:], xln1[:, qt * D:(qt + 1) * D],
                          g1_b[:])
                for bd in range(NB):
                    tp = tpp.tile([P, P], FP32, tag="tp")
                    nc.tensor.transpose(
                        tp[:], xln1[:, qt * D + bd * P: qt * D + (bd + 1) * P],
                        ident[:])
                    # copy on Act (idle here; DVE is the LN bottleneck) and
                    # apply beta1 -- per-partition in transposed space
                    nc.scalar.activation(
                        xln1T[:, bd * TPC + qt * P: bd * TPC + (qt + 1) * P],
                        tp[:], AF.Identity, bias=be1_c[:, bd:bd + 1])
                _fill(8)

        # ot / wp / xbp dead; free their space for the FFN tensors
        proj_era.close()

        bc2 = top.enter_context(tc.tile_pool(name="bc2", bufs=1))
        b2_b = bc2.tile([P, D], FP32, name="b2b")
        nc.sync.dma_start(b2_b[:], b2_d.ap()[None, :].to_broadcast((P, D)))
        nc.vector.tensor_add(b2_b[:], b2_b[:], be1_b[:])
        g2_b = bc2.tile([P, D], FP32, name="g2b")
        nc.sync.dma_start(g2_b[:], g2_d.ap()[None, :].to_broadcast((P, D)))
        be2_b = bc2.tile([P, D], FP32, name="be2b")
        nc.sync.dma_start(be2_b[:], be2_d.ap()[None, :].to_broadcast((P, D)))

        # ================= FFN =================
        ffw = top.enter_context(tc.tile_pool(name="ffw", bufs=1))
        hT = ffw.tile([P, FB * TPC], BF16)        # 32KB/part
        w2_sb = ffw.tile([P, FB * D], BF16)       # 64KB/part
        for c4 in range(4):
            nc.scalar.dma_start(
                w2_sb[:, c4 * 8 * D:(c4 + 1) * 8 * D].rearrange(
                    "p (k n) -> p k n", n=D),
                w2_d.ap()[c4 * 8 * P:(c4 + 1) * 8 * P, :].rearrange(
                    "(k p) n -> p k n", p=P))
        with (
            tc.tile_pool(name="w1s", bufs=6) as w1s,
            tc.tile_pool(name="fps", bufs=3, space="PSUM") as fps,
            tc.tile_pool(name="f2ps", bufs=4, space="PSUM") as f2ps,
            tc.tile_pool(name="flp2", bufs=1, space="PSUM") as flp2,
            tc.tile_pool(name="fscr", bufs=2) as fscr,
        ):
            # FFN1 in two q-halves: half 0 only needs q-tiles 0,1 of
            # xln1T, so its matmuls fill the PE gap while LN1 of q-tiles
            # 2,3 is still running on DVE/Act. w1 is streamed twice.
            flw2 = fscr.tile([P, TPC], BF16, tag="flw2")
            nc.vector.memset(flw2[:], 0.0)
            flq2 = flp2.tile([P, TPC], FP32, tag="flq2", name="fill2_ps")
            HQ = TPC // 2
            for half in range(2):
                if half == 1:
                    for _ in range(18):
                        nc.tensor.matmul(flq2[:], lhsT=flw2[:, 0:P],
                                         rhs=flw2[:], start=True, stop=True)
                for mf in range(FB):
                    w1t = w1s.tile([P, KB * P], BF16, tag="w1t")
                    nc.sync.dma_start(
                        w1t[:].rearrange("p (k c) -> p k c", c=P),
                        w1_d.ap()[:, mf * P:(mf + 1) * P].rearrange(
                            "(k p) c -> p k c", p=P))
                    ph = fps.tile([P, HQ], FP32, tag="ph")
                    for k in range(KB):
                        nc.tensor.matmul(
                            ph[:],
                            lhsT=w1t[:, k * P:(k + 1) * P],
                            rhs=xln1T[:, k * TPC + half * HQ:
                                      k * TPC + (half + 1) * HQ],
                            start=(k == 0), stop=(k == KB - 1),
                        )
                    nc.scalar.activation(
                        hT[:, mf * TPC + half * HQ:
                           mf * TPC + (half + 1) * HQ], ph[:], AF.Relu,
                        bias=b1_c[:, mf:mf + 1])

            for qt in range(QT):
                pj2 = [f2ps.tile([P, TPC], FP32, tag="pj2",
                                 name=f"pj2_{qt}_{nd}") for nd in range(2)]
                for k2 in range(FB):
                    for nd in range(2):
                        nc.tensor.matmul(
                            pj2[nd][:],
                            lhsT=hT[:, k2 * TPC + qt * P:
                                    k2 * TPC + (qt + 1) * P],
                            rhs=w2_sb[:, k2 * D + nd * TPC:
                                      k2 * D + (nd + 1) * TPC],
                            start=(k2 == 0), stop=(k2 == FB - 1),
                        )
                y2 = fscr.tile([P, D], FP32, tag="y2")
                for nd in range(2):
                    nc.vector.tensor_add(
                        y2[:, nd * TPC:(nd + 1) * TPC], pj2[nd][:],
                        xln1[:, qt * D + nd * TPC: qt * D + (nd + 1) * TPC])
                nc.vector.tensor_add(y2[:], y2[:], b2_b[:])
                yo = fscr.tile([P, D], FP32, tag="yo")
                _ln_chain(nc, fscr, y2[:], yo[:], g2_b[:], be2_b[:])
                nc.sync.dma_start(out_d.ap()[qt * P:(qt + 1) * P, :], yo[:])


def _attn_heads(nc, use_mask, h0, h1, kt_sb, qt_sb, vp_sb, ot_sb,
                ascr, sps, ops, dramp, mk_pool, maskT_d, identb):
    """Attention for heads [h0, h1) -- emitted interleaved with the K
    projection so the PE stream stays dense."""
    for h in range(h0, h1):
        m, hp = h // 2, (h % 2) * DH
        op = ops.tile([DH + 1, TPC], FP32, tag="op")
        for kp in range(KT16 // 2):
            # two kpos tiles share one 2-bank psum tile; ONE exp covers
            # both -- Act's per-instruction overhead was the attention
            # pacer, so halving instruction count shortens the period
            sp = sps.tile([P, 2 * TPC], FP32, tag="sp")
            if use_mask:
                mk = mk_pool.tile([P, 2 * TPC], BF16, tag="mk")
                for sub in range(2):
                    kt = 2 * kp + sub
                    nc.sync.dma_start(mk[:, sub * TPC:(sub + 1) * TPC],
                                      maskT_d.ap()[kt * P:(kt + 1) * P, :])
            for sub in range(2):
                kt = 2 * kp + sub
                nc.tensor.matmul(
                    sp[:, sub * TPC:(sub + 1) * TPC],
                    lhsT=kt_sb[hp:hp + DH,
                               m * S + kt * P: m * S + (kt + 1) * P],
                    rhs=qt_sb[hp:hp + DH, m * TPC:(m + 1) * TPC],
                    start=True, stop=(not use_mask),
                )
                if use_mask:
                    nc.tensor.matmul(
                        sp[:, sub * TPC:(sub + 1) * TPC],
                        lhsT=identb[:],
                        rhs=mk[:, sub * TPC:(sub + 1) * TPC],
                        start=False, stop=True,
                    )
            et = ascr.tile([P, 2 * TPC], BF16, tag="et")
            nc.scalar.activation(et[:], sp[:], AF.Exp, scale=SCALE)
            for sub in range(2):
                kt = 2 * kp + sub
                nc.tensor.matmul(
                    op[:],
                    lhsT=vp_sb[:, kt * VW + h * (DH + 1):
                               kt * VW + (h + 1) * (DH + 1)],
                    rhs=et[:, sub * TPC:(sub + 1) * TPC],
                    start=(kt == 0), stop=(kt == KT16 - 1),
                )
        rr = ascr.tile([1, TPC], FP32, tag="rr")
        if h >= H - 4:
            # tail heads: Act is idle (no more exps queued) while DVE's
            # 3.35us reciprocal would gate the attention->proj handoff;
            # 1/x = exp(-ln(x)) from the one loaded act table
            lt = ascr.tile([1, TPC], FP32, tag="rlt")
            nc.scalar.activation(lt[:], op[DH:DH + 1, :], AF.Ln)
            nc.scalar.activation(rr[:], lt[:], AF.Exp, scale=-1.0)
        else:
            nc.vector.reciprocal(rr[:], op[DH:DH + 1, :])
        # broadcast rr along partitions: SBUF sources can't have stride-0
        # partition APs, so bounce the 2KB row through DRAM (idle queues)
        rr_d = dramp.tile([1, TPC], FP32, tag="rrd", name=f"rr_{h}")
        nc.sync.dma_start(rr_d[:], rr[:])
        rb = ascr.tile([DH, TPC], FP32, tag="rb")
        nc.sync.dma_start(rb[:], rr_d[:1, :].to_broadcast((DH, TPC)))
        with nc.allow_low_precision(reason="bf16 attention output"):
            nc.vector.tensor_mul(
                ot_sb[hp:hp + DH, m * TPC:(m + 1) * TPC],
                op[0:DH, :], rb[:])


_PROG_CACHE: dict = {}


def _get_program(use_mask: bool) -> bass.Bass:
    if use_mask not in _PROG_CACHE:
        _PROG_CACHE[use_mask] = build_program(use_mask)
    return _PROG_CACHE[use_mask]


def make_in_maps(x, mask, Wq, bq, Wk, bk, Wv, bv, Wp, bp,
                 gamma1, beta1, W1, b1, W2, b2, gamma2, beta2):
    bf = mybir.dt.np(BF16)
    x = np.asarray(x, np.float32)
    mask = np.asarray(mask)
    use_mask = not bool(mask.all())
    bpp = (np.asarray(bp, np.float32)
           + np.asarray(bv, np.float32) @ np.asarray(Wp, np.float32))
    common = {
        "wq": np.ascontiguousarray(Wq).astype(bf),
        "wk": np.ascontiguousarray(Wk).astype(bf),
        "wv": np.ascontiguousarray(Wv).astype(bf),
        "wp": np.ascontiguousarray(Wp).astype(bf),
        "w1": np.ascontiguousarray(W1).astype(bf),
        "w2": np.ascontiguousarray(W2).astype(bf),
        "bq": np.ascontiguousarray(bq, np.float32),
        "bk": np.ascontiguousarray(bk, np.float32),
        "b1": np.ascontiguousarray(b1, np.float32),
        "b2": np.ascontiguousarray(b2, np.float32),
        "g1": np.ascontiguousarray(gamma1, np.float32),
        "be1": np.ascontiguousarray(beta1, np.float32),
        "g2": np.ascontiguousarray(gamma2, np.float32),
        "be2": np.ascontiguousarray(beta2, np.float32),
    }
    if use_mask:
        mbias = np.where(mask, np.float32(0.0), np.float32(-1e12))
        # mask is added to raw scores, but exp applies SCALE to its input
        # (exp(SCALE*(s+m))); reference adds mask after scaling -- rescale.
        mbias = (mbias / np.float32(SCALE)).astype(bf)
    in_maps = []
    for c in range(N_CORES):
        b, j = divmod(c, 4)
        xb = x[b]
        m = dict(common)
        m["xT"] = np.ascontiguousarray(xb.T).astype(bf)
        m["xTq"] = np.ascontiguousarray(xb[j * TPC:(j + 1) * TPC].T).astype(bf)
        m["xbp"] = np.ascontiguousarray(xb[j * TPC:(j + 1) * TPC] + bpp)
        if use_mask:
            m["maskT"] = np.ascontiguousarray(mbias.T[:, j * TPC:(j + 1) * TPC])
        in_maps.append(m)
    return use_mask, in_maps


def assemble_output(results) -> np.ndarray:
    out = np.empty((2, S, D), np.float32)
    for c in range(N_CORES):
        b, j = divmod(c, 4)
        out[b, j * TPC:(j + 1) * TPC] = results[c]["out"]
    return out


def kernel(**inputs) -> np.ndarray:
    use_mask, in_maps = make_in_maps(**inputs)
    nc = _get_program(use_mask)
    res = run_bass_kernel_spmd(nc, in_maps, list(range(N_CORES)))
    return assemble_output(res.results)
